# revision 9
# baseline (speedup 1.0000x reference)
"""CrossAttnBlock Trainium2 kernel (8 NeuronCores, data-parallel over batch).

Contract: kernel(**inputs) takes the FULL unsharded inputs (as produced by
setup_inputs) and returns the FULL (192, 64, 384) float32 output.

Per-core dataflow (24 batches/core):
  - x LayerNorm (token-major, bn_stats) -> xn bf16 -> XBAR transpose -> xnT
  - Q = xn @ Wq (head-padded to 64-wide slots) -> qT feature-major
  - per batch: context LN -> cnT (XBAR transpose); K proj (head-padded,
    feature-major); V proj (token-major); sim via 2-head row-packed matmuls;
    fused Exp(scale=mask*SCALE, accum_out=rowsum) straight from PSUM;
    p transposed via XBAR; attn@v token-major; 1/Z as per-partition scale;
    o transposed; out-proj + residual (fp32)
  - norm2 -> MLP1 (+ fused Gelu_apprx_tanh) -> MLP2 + residual -> out

Host-side prep is limited to slicing/reshaping inputs, casting weights to
bf16, and folding LayerNorm affine params / head padding into the weights.
"""
import numpy as np
import ml_dtypes

import concourse.bass as bass
import concourse.mybir as mybir
import concourse.tile as tile
from concourse import bacc
from concourse.bass_utils import run_bass_kernel_spmd

BT, NQ, NCTX, C, H, DH, HID = 192, 64, 1024, 384, 8, 48, 1536
NCORES = 8
BL = BT // NCORES            # 24 local batches per core
TOK = BL * NQ                # 1536 local query tokens
SCALE = DH ** -0.5
P = 128
DHP = 64                     # padded per-head slot
FQK = H * DHP                # 512 padded q/k feature dim
KC = C // P                  # 3 contraction chunks of C
FC_QK = FQK // P             # 4 feature chunks of padded q/k
TCH = NCTX // P              # 8 context token chunks per batch
XT = TOK // P                # 12 query token chunks per core
HC = HID // P                # 12 hidden chunks

BF16 = mybir.dt.bfloat16
F32 = mybir.dt.float32
AF = mybir.ActivationFunctionType
OP = mybir.AluOpType

_CACHED_NC = None


def _build():
    nc = bacc.Bacc("TRN2", target_bir_lowering=False, debug=False,
                   num_devices=NCORES)

    d_x = nc.dram_tensor("x", [TOK, C], F32, kind="ExternalInput").ap()
    d_ctx = nc.dram_tensor("ctx", [BL, NCTX, C], F32, kind="ExternalInput").ap()
    d_mask = nc.dram_tensor("maskscale", [NQ, BL], F32, kind="ExternalInput").ap()
    d_wq = nc.dram_tensor("wq", [C, FQK], BF16, kind="ExternalInput").ap()
    d_bq = nc.dram_tensor("bq", [FQK], F32, kind="ExternalInput").ap()
    d_wk = nc.dram_tensor("wk", [C, FQK], BF16, kind="ExternalInput").ap()
    d_bk = nc.dram_tensor("bk", [FQK], F32, kind="ExternalInput").ap()
    d_wv = nc.dram_tensor("wv", [C, C], BF16, kind="ExternalInput").ap()
    d_bv = nc.dram_tensor("bv", [C], F32, kind="ExternalInput").ap()
    d_wo = nc.dram_tensor("wo", [C, C], BF16, kind="ExternalInput").ap()
    d_bo = nc.dram_tensor("bo", [C], F32, kind="ExternalInput").ap()
    d_w1 = nc.dram_tensor("w1", [C, HID], BF16, kind="ExternalInput").ap()
    d_b1 = nc.dram_tensor("b1", [HID], F32, kind="ExternalInput").ap()
    d_w2 = nc.dram_tensor("w2", [HID, C], BF16, kind="ExternalInput").ap()
    d_b2 = nc.dram_tensor("b2", [C], F32, kind="ExternalInput").ap()
    d_out = nc.dram_tensor("out", [TOK, C], F32, kind="ExternalOutput").ap()

    with tile.TileContext(nc) as tc:
        _prog(nc, tc, d_x, d_ctx, d_mask, d_wq, d_bq, d_wk, d_bk, d_wv, d_bv,
              d_wo, d_bo, d_w1, d_b1, d_w2, d_b2, d_out)
    nc.compile()
    return nc


def _bcast_load(nc, pool, dram_vec, n):
    """Load a [n] DRAM vector broadcast across all 128 partitions."""
    t = pool.tile([P, n], F32)
    src = bass.AP(tensor=dram_vec.tensor, offset=dram_vec.offset,
                  ap=[[0, P]] + list(dram_vec.ap))
    nc.gpsimd.dma_start(out=t, in_=src)
    return t


def _ln_chunk(nc, pool, raw, eps, out_bf16):
    """LayerNorm a [p, C] chunk (stats over free dim) -> bf16 out."""
    pdim = raw.shape[0]
    stats = pool.tile([P, 6], F32, tag="ln_stats")
    nc.vector.bn_stats(stats[:pdim], raw)
    mv = pool.tile([P, 2], F32, tag="ln_mv")
    nc.vector.bn_aggr(mv[:pdim], stats[:pdim])
    rstd = pool.tile([P, 1], F32, tag="ln_rstd")
    nc.scalar.activation(rstd[:pdim], mv[:pdim, 1:2], AF.Sqrt, bias=eps[:pdim])
    nc.vector.reciprocal(rstd[:pdim], rstd[:pdim])
    nc.vector.tensor_scalar(out=out_bf16, in0=raw, scalar1=mv[:pdim, 0:1],
                            scalar2=rstd[:pdim], op0=OP.subtract, op1=OP.mult)


def _prog(nc, tc, d_x, d_ctx, d_mask, d_wq, d_bq, d_wk, d_bk, d_wv, d_bv,
          d_wo, d_bo, d_w1, d_b1, d_w2, d_b2, d_out):
    with (tc.tile_pool(name="singles", bufs=1) as singles,
          tc.tile_pool(name="big", bufs=1) as big,
          tc.tile_pool(name="lnp", bufs=4) as lnp,
          tc.tile_pool(name="xtmp", bufs=3) as xtmp,
          tc.tile_pool(name="ctxp", bufs=3) as ctxp,
          tc.tile_pool(name="cnp", bufs=2) as cnp,
          tc.tile_pool(name="batchp", bufs=2) as batchp,
          tc.tile_pool(name="smallp", bufs=3) as smallp,
          tc.tile_pool(name="outp", bufs=2) as outp,
          tc.tile_pool(name="ps", bufs=4, space="PSUM") as ps,
          tc.tile_pool(name="ps_o", bufs=2, space="PSUM") as ps_o,
          tc.tile_pool(name="ps_x", bufs=2, space="PSUM") as ps_x):
        _body(nc, tc, singles, big, lnp, xtmp, ctxp, cnp, batchp, smallp,
              outp, ps, ps_o, ps_x, d_x, d_ctx, d_mask, d_wq, d_bq, d_wk,
              d_bk, d_wv, d_bv, d_wo, d_bo, d_w1, d_b1, d_w2, d_b2, d_out)


def _body(nc, tc, singles, big, lnp, xtmp, ctxp, cnp, batchp, smallp, outp,
          ps, ps_o, ps_x, d_x, d_ctx, d_mask, d_wq, d_bq, d_wk, d_bk, d_wv,
          d_bv, d_wo, d_bo, d_w1, d_b1, d_w2, d_b2, d_out):
    # ---- load weights / consts ----
    wq_sb = singles.tile([P, KC, FQK], BF16)
    nc.sync.dma_start(wq_sb, d_wq.rearrange("(kc p) f -> p kc f", p=P))
    wk_sb = singles.tile([P, KC, FQK], BF16)
    nc.sync.dma_start(wk_sb, d_wk.rearrange("(kc p) f -> p kc f", p=P))
    wv_sb = singles.tile([P, KC, C], BF16)
    nc.sync.dma_start(wv_sb, d_wv.rearrange("(kc p) f -> p kc f", p=P))
    wo_sb = singles.tile([P, KC, C], BF16)
    nc.sync.dma_start(wo_sb, d_wo.rearrange("(kc p) f -> p kc f", p=P))
    w1_sb = singles.tile([P, KC, HID], BF16)
    nc.sync.dma_start(w1_sb, d_w1.rearrange("(kc p) f -> p kc f", p=P))
    w2_sb = singles.tile([P, HC, C], BF16)
    nc.sync.dma_start(w2_sb, d_w2.rearrange("(kc p) f -> p kc f", p=P))
    bq_sb = singles.tile([P, FC_QK], F32)
    nc.sync.dma_start(bq_sb, d_bq.rearrange("(fc p) -> p fc", p=P))
    bk_sb = singles.tile([P, FC_QK], F32)
    nc.sync.dma_start(bk_sb, d_bk.rearrange("(fc p) -> p fc", p=P))
    b1_sb = singles.tile([P, HC], F32)
    nc.sync.dma_start(b1_sb, d_b1.rearrange("(fc p) -> p fc", p=P))
    bv_bc = _bcast_load(nc, singles, d_bv, C)
    bo_bc = _bcast_load(nc, singles, d_bo, C)
    b2_bc = _bcast_load(nc, singles, d_b2, C)
    mask_sb = singles.tile([NQ, BL], F32)
    nc.sync.dma_start(mask_sb, d_mask)
    eps6 = singles.tile([P, 1], F32)
    nc.vector.memset(eps6, 1e-6)
    eps5 = singles.tile([P, 1], F32)
    nc.vector.memset(eps5, 1e-5)

    # ---- x LayerNorm (norm1) + transpose ----
    x_sb = big.tile([P, XT, C], F32, tag="bigA")           # raw x, resident
    nc.sync.dma_start(x_sb, d_x.rearrange("(t p) c -> p t c", p=P))
    xnT = big.tile([P, XT, KC, P], BF16, tag="xT")         # chunk-major
    for t in range(XT):
        xn = xtmp.tile([P, C], BF16, tag="xn")
        _ln_chunk(nc, lnp, x_sb[:, t, :], eps6, xn)
        nc.sync.dma_start_transpose(xnT[:, t], xn)

    # ---- Q projection -> qT [128, FC_QK, TOK] bf16 ----
    qT = big.tile([P, FC_QK, TOK], BF16, tag="qT")
    for fc in range(FC_QK):
        for n in range(TOK // 512):
            pq = ps.tile([P, 512], F32, tag="big")
            for kc in range(KC):
                nc.tensor.matmul(pq, lhsT=wq_sb[:, kc, fc * P:(fc + 1) * P],
                                 rhs=xnT[:, 4 * n:4 * n + 4, kc, :],
                                 start=(kc == 0), stop=(kc == KC - 1))
            nc.vector.tensor_scalar(out=qT[:, fc, n * 512:(n + 1) * 512],
                                    in0=pq, scalar1=bq_sb[:, fc:fc + 1],
                                    scalar2=None, op0=OP.add)

    x2_sb = big.tile([P, XT, C], F32, tag="x2")            # residual stream

    # ---- per-batch context/attention ----
    for b in range(BL):
        # context LN, streamed per 128-token chunk
        cnT = batchp.tile([P, TCH, KC, P], BF16, tag="cnT")
        for t in range(TCH):
            craw = ctxp.tile([P, C], F32, tag="craw")
            nc.sync.dma_start(craw, d_ctx[b, t * P:(t + 1) * P, :])
            cn = cnp.tile([P, C], BF16, tag="cn")
            _ln_chunk(nc, lnp, craw, eps5, cn)
            nc.sync.dma_start_transpose(cnT[:, t], cn)

        # K projection (padded heads) -> kT [128, FC_QK, NCTX]
        kT = batchp.tile([P, FC_QK, NCTX], BF16, tag="kT")
        for fc in range(FC_QK):
            for n in range(NCTX // 512):
                pk = ps.tile([P, 512], F32, tag="big")
                for kc in range(KC):
                    nc.tensor.matmul(pk, lhsT=wk_sb[:, kc, fc * P:(fc + 1) * P],
                                     rhs=cnT[:, 4 * n:4 * n + 4, kc, :],
                                     start=(kc == 0), stop=(kc == KC - 1))
                nc.vector.tensor_scalar(out=kT[:, fc, n * 512:(n + 1) * 512],
                                        in0=pk, scalar1=bk_sb[:, fc:fc + 1],
                                        scalar2=None, op0=OP.add)

        # V projection, token-major -> v_tok [128, TCH, C]
        v_tok = batchp.tile([P, TCH, C], BF16, tag="v")
        for t in range(TCH):
            pv = ps.tile([P, C], F32, tag="big")
            for kc in range(KC):
                nc.tensor.matmul(pv, lhsT=cnT[:, t, kc, :], rhs=wv_sb[:, kc, :],
                                 start=(kc == 0), stop=(kc == KC - 1))
            nc.vector.tensor_tensor(v_tok[:, t, :], pv, bv_bc, OP.add)

        # sim + fused masked exp with row-sum accumulation
        # p packed two heads per 128 partitions: head 2j+half on rows
        # [64*half, 64*half+64) of chunk j
        p_sb = batchp.tile([P, H // 2, NCTX], BF16, tag="p")
        sums = smallp.tile([NQ, H, 2], F32, tag="sums")
        for j in range(H // 2):
            for n in range(NCTX // 512):
                pp = [ps.tile([NQ, 512], F32, tag="big", name=f"simps{h2}")
                      for h2 in range(2)]
                for half in range(2):
                    base = DHP * half
                    nc.tensor.matmul(
                        pp[half],
                        lhsT=qT[base:base + DH, j, b * NQ:(b + 1) * NQ],
                        rhs=kT[base:base + DH, j, n * 512:(n + 1) * 512],
                        start=True, stop=True)
                for half in range(2):
                    h = 2 * j + half
                    nc.scalar.activation(
                        p_sb[NQ * half:NQ * half + NQ, j,
                             n * 512:(n + 1) * 512], pp[half], AF.Exp,
                        scale=mask_sb[:, b:b + 1],
                        accum_out=sums[:, h, n:n + 1])
        zr = smallp.tile([NQ, H], F32, tag="zr")
        nc.vector.tensor_tensor(zr, sums[:, :, 0], sums[:, :, 1], OP.add)
        nc.vector.reciprocal(zr, zr)

        # transpose p -> pT [128, H, TCH, NQ]
        pT = batchp.tile([P, H, TCH, NQ], BF16, tag="pT")
        for h in range(H):
            j, half = h // 2, h % 2
            nc.sync.dma_start_transpose(
                pT[:, h], p_sb[NQ * half:NQ * half + NQ, j, :])

        # attn @ v, token-major out [NQ, H, DH]; fold 1/Z on eviction
        o_tok = smallp.tile([NQ, H, DH], BF16, tag="otok")
        po = ps_o.tile([NQ, H, DH], F32, tag="attnv")
        for h in range(H):
            for t in range(TCH):
                nc.tensor.matmul(po[:, h, :], lhsT=pT[:, h, t, :],
                                 rhs=v_tok[:, t, h * DH:(h + 1) * DH],
                                 start=(t == 0), stop=(t == TCH - 1))
        for h in range(H):
            nc.vector.tensor_scalar(out=o_tok[:, h, :], in0=po[:, h, :],
                                    scalar1=zr[:, h:h + 1], scalar2=None,
                                    op0=OP.mult)
        oT = smallp.tile([P, KC, NQ], BF16, tag="oT")
        nc.sync.dma_start_transpose(oT, o_tok)

        # out projection + residual -> x2
        px = ps_x.tile([NQ, C], F32, tag="oproj")
        for kc in range(KC):
            nc.tensor.matmul(px, lhsT=oT[:, kc, :], rhs=wo_sb[:, kc, :],
                             start=(kc == 0), stop=(kc == KC - 1))
        r = b % 2
        x2_dst = x2_sb[r * NQ:(r + 1) * NQ, b // 2, :]
        nc.vector.tensor_copy(x2_dst, px)
        nc.vector.tensor_tensor(x2_dst, x2_dst,
                                x_sb[r * NQ:(r + 1) * NQ, b // 2, :], OP.add)
        nc.vector.tensor_tensor(x2_dst, x2_dst, bo_bc[r * NQ:(r + 1) * NQ],
                                OP.add)

    # ---- norm2 + transpose ----
    xn2T = big.tile([P, XT, KC, P], BF16, tag="xT")        # reuses xnT slot
    for t in range(XT):
        xn2 = xtmp.tile([P, C], BF16, tag="xn")
        _ln_chunk(nc, lnp, x2_sb[:, t, :], eps6, xn2)
        nc.sync.dma_start_transpose(xn2T[:, t], xn2)

    # ---- MLP1 with fused tanh-GELU -> hT [128, HC, TOK] ----
    hT = big.tile([P, HC, TOK], BF16, tag="bigA")          # reuses x_sb slot
    for fc in range(HC):
        for n in range(TOK // 512):
            ph = ps.tile([P, 512], F32, tag="big")
            for kc in range(KC):
                nc.tensor.matmul(ph, lhsT=w1_sb[:, kc, fc * P:(fc + 1) * P],
                                 rhs=xn2T[:, 4 * n:4 * n + 4, kc, :],
                                 start=(kc == 0), stop=(kc == KC - 1))
            nc.scalar.activation(hT[:, fc, n * 512:(n + 1) * 512], ph,
                                 AF.Gelu_apprx_tanh, bias=b1_sb[:, fc:fc + 1])

    # ---- MLP2 + residual -> out ----
    for t in range(XT):
        pm = ps.tile([P, C], F32, tag="big")
        for kc in range(HC):
            nc.tensor.matmul(pm, lhsT=hT[:, kc, t * P:(t + 1) * P],
                             rhs=w2_sb[:, kc, :],
                             start=(kc == 0), stop=(kc == HC - 1))
        yt = outp.tile([P, C], F32, tag="y")
        nc.vector.tensor_tensor(yt, pm, x2_sb[:, t, :], OP.add)
        nc.vector.tensor_tensor(yt, yt, b2_bc, OP.add)
        nc.sync.dma_start(d_out[t * P:(t + 1) * P, :], yt)


def _get_nc():
    global _CACHED_NC
    if _CACHED_NC is None:
        _CACHED_NC = _build()
    return _CACHED_NC


def _pad_heads(w):
    """[C, H*DH] -> [C, H*DHP] with each head's DH cols at 64-aligned slots."""
    out = np.zeros((w.shape[0], FQK), w.dtype)
    for h in range(H):
        out[:, h * DHP:h * DHP + DH] = w[:, h * DH:(h + 1) * DH]
    return out


def _pad_heads_vec(v):
    out = np.zeros((FQK,), v.dtype)
    for h in range(H):
        out[h * DHP:h * DHP + DH] = v[h * DH:(h + 1) * DH]
    return out


def prep_in_maps(x, context, mask, Wq, bq, Wkv, bkv, Wo, bo, g_ctx, b_ctx,
                 W1, b1, W2, b2):
    x = np.asarray(x, np.float32)
    context = np.asarray(context, np.float32)
    mask = np.asarray(mask)
    f32 = lambda a: np.asarray(a, np.float32)
    Wq, bq, Wkv, bkv = f32(Wq), f32(bq), f32(Wkv), f32(bkv)
    Wo, bo, g_ctx, b_ctx = f32(Wo), f32(bo), f32(g_ctx), f32(b_ctx)
    W1, b1, W2, b2 = f32(W1), f32(b1), f32(W2), f32(b2)

    # fold context-LN affine into Wkv / bkv
    wkv_g = g_ctx[:, None] * Wkv
    bkv_eff = b_ctx @ Wkv + bkv
    bf = lambda a: np.ascontiguousarray(a.astype(ml_dtypes.bfloat16))
    shared = {
        "wq": bf(_pad_heads(Wq)), "bq": _pad_heads_vec(bq),
        "wk": bf(_pad_heads(wkv_g[:, :C])), "bk": _pad_heads_vec(bkv_eff[:C]),
        "wv": bf(wkv_g[:, C:]), "bv": np.ascontiguousarray(bkv_eff[C:]),
        "wo": bf(Wo), "bo": bo,
        "w1": bf(W1), "b1": b1, "w2": bf(W2), "b2": b2,
    }
    in_maps = []
    for c in range(NCORES):
        sl = slice(c * BL, (c + 1) * BL)
        in_maps.append({
            "x": np.ascontiguousarray(x[sl].reshape(TOK, C)),
            "ctx": np.ascontiguousarray(context[sl]),
            "maskscale": np.ascontiguousarray(
                (mask[sl].T.astype(np.float32) * SCALE)),
            **shared,
        })
    return in_maps


def kernel(**inputs):
    in_maps = prep_in_maps(**inputs)
    nc = _get_nc()
    res = run_bass_kernel_spmd(nc, in_maps, core_ids=list(range(NCORES)))
    out = np.concatenate(
        [res.results[c]["out"].reshape(BL, NQ, C) for c in range(NCORES)], 0)
    return out
